# revision 4
# baseline (speedup 1.0000x reference)
"""Trainium2 Bass kernel for nn_BottleneckBlock (half-channel causal attention).

Contract: kernel(**inputs) takes the FULL unsharded inputs (as produced by the
problem's setup_inputs) and returns the FULL output, distributing work across
8 NeuronCores data-parallel over the (b, h, w) positions.

Per-core math (144 positions, seq N=64, C=256, 8 heads x 32):
  qkv = x @ qkv_w.T + qkv_b ; causal softmax(q k^T / sqrt(32) + rpb) @ v ; proj.
Layout strategy: everything token-transposed ([channel, token] in SBUF) so that
matmul contractions always run along partitions. The softmax bias+mask is
applied multiplicatively as exp(bias) (0 above the diagonal); the softmax
denominator comes out of the attention-value matmul via an appended
all-ones column; V's projection bias is folded into the output projection
bias (b_out' = proj_b + proj_w @ b_v) on the host.
"""

import os
import sys
from contextlib import ExitStack

import numpy as np

sys.path.insert(0, "/opt/trn_rl_repo")

import ml_dtypes

BF16 = ml_dtypes.bfloat16

# Problem shape (hardcoded per spec)
B, T, CH, HS, WS = 2, 64, 512, 24, 24
HALF = CH // 2          # 256
HEADS = 8
HD = 32
SCALE = HD ** -0.5
NCORES = 8
NPOS = B * HS * WS      # 1152 positions
POS_PER_CORE = NPOS // NCORES   # 144
TOK = POS_PER_CORE * T  # 9216 tokens per core
ROUNDS = TOK // 512     # 18 rounds of 8 positions

_BUILD_CACHE = {}


def _bf(a):
    return np.ascontiguousarray(a.astype(BF16))


def _host_prep(x, rpb_table, qkv_w, qkv_b, proj_w, proj_b):
    """Build the 8 per-core input maps + keep what's needed for reassembly."""
    x = np.asarray(x, dtype=np.float32)
    qkv_w = np.asarray(qkv_w, dtype=np.float32)
    qkv_b = np.asarray(qkv_b, dtype=np.float32)
    proj_w = np.asarray(proj_w, dtype=np.float32)
    proj_b = np.asarray(proj_b, dtype=np.float32)
    rpb = np.asarray(rpb_table, dtype=np.float32)

    # ---- x transpose: (B,T,CH,H,W) attention half -> [c, B_*T] ----
    b_part = x[:, :, HALF:]                       # (B,T,256,H,W)
    xt = np.transpose(b_part, (2, 0, 3, 4, 1))    # (256, B, H, W, T)
    xt = np.ascontiguousarray(xt).reshape(HALF, NPOS * T)
    xt_bf = xt.astype(BF16)

    # ---- weights ----
    # wqk[p, kc, f]: f in [0,512) = (q heads then k heads); q rows pre-scaled
    wqk = np.empty((128, 2, 512), dtype=np.float32)
    for kc in range(2):
        wqk[:, kc, 0:256] = (qkv_w[0:256] * SCALE).T[128 * kc: 128 * (kc + 1)]
        wqk[:, kc, 256:512] = qkv_w[256:512].T[128 * kc: 128 * (kc + 1)]
    bqk = np.stack(
        [qkv_b[0:128] * SCALE, qkv_b[128:256] * SCALE,
         qkv_b[256:384], qkv_b[384:512]], axis=1,
    ).astype(np.float32)                          # [128, 4]

    wv = np.empty((128, 2, 256), dtype=np.float32)
    for kc in range(2):
        wv[:, kc, :] = qkv_w[512:768].T[128 * kc: 128 * (kc + 1)]

    wp = np.empty((128, 2, 256), dtype=np.float32)
    for w in range(2):
        wp[:, w, :] = proj_w.T[128 * w: 128 * (w + 1)]
    bv = qkv_b[512:768]
    bp_full = proj_b + proj_w @ bv
    bp = np.stack([bp_full[0:128], bp_full[128:256]], axis=1).astype(np.float32)

    # ---- exp(bias)+causal-mask tiles, transposed & replicated ----
    pos = np.arange(T)
    rel = pos[None, :] - pos[:, None] + (T - 1)   # [i, j]
    bias = rpb[rel]                               # [i, j, heads]
    eb = np.exp(bias.transpose(2, 0, 1))          # [h, i, j]
    eb = eb * (pos[None, None, :] <= pos[None, :, None])  # zero j>i
    ebT = np.transpose(eb, (0, 2, 1))             # [h, j, i]
    ebrep = np.empty((128, 4, 512), dtype=np.float32)
    for r in range(4):
        for q in range(2):
            for slot in range(8):
                hh = slot // 4
                ebrep[64 * q: 64 * (q + 1), r, 64 * slot: 64 * (slot + 1)] = ebT[r + 4 * hh]

    wqk_bf, wv_bf, wp_bf, ebrep_bf = _bf(wqk), _bf(wv), _bf(wp), _bf(ebrep)

    in_maps = []
    for m in range(NCORES):
        sl = xt_bf[:, m * TOK: (m + 1) * TOK]     # [256, 9216]
        in_maps.append({
            "xT": np.ascontiguousarray(sl.reshape(2, 128, TOK)),
            "wqk": wqk_bf,
            "wv": wv_bf,
            "wp": wp_bf,
            "bqk": bqk,
            "bp": bp,
            "ebrep": ebrep_bf,
        })
    return in_maps


def _emit(nc, tc, d):
    """Emit the Tile kernel. d: dict of dram APs."""
    import concourse.bass as bass
    import concourse.mybir as mybir

    f32 = mybir.dt.float32
    bf16 = mybir.dt.bfloat16
    AFT = mybir.ActivationFunctionType

    ctx = tc._emit_ctx  # ExitStack installed by caller

    consts = ctx.enter_context(tc.tile_pool(name="consts", bufs=1))
    persist = ctx.enter_context(tc.tile_pool(name="persist", bufs=1))
    sb = ctx.enter_context(tc.tile_pool(name="sb", bufs=3))
    sb_qk = ctx.enter_context(tc.tile_pool(name="sb_qk", bufs=3))
    sb_ep = ctx.enter_context(tc.tile_pool(name="sb_ep", bufs=8))
    ps_mm = ctx.enter_context(tc.tile_pool(name="ps_mm", bufs=2, space="PSUM"))
    ps_s = ctx.enter_context(tc.tile_pool(name="ps_s", bufs=1, space="PSUM"))
    ps_av = ctx.enter_context(tc.tile_pool(name="ps_av", bufs=2, space="PSUM"))

    # ---- load constants ----
    wqk_sb = consts.tile([128, 2, 512], bf16)
    nc.sync.dma_start(wqk_sb, d["wqk"])
    wv_sb = consts.tile([128, 2, 256], bf16)
    nc.sync.dma_start(wv_sb, d["wv"])
    wp_sb = consts.tile([128, 2, 256], bf16)
    nc.sync.dma_start(wp_sb, d["wp"])
    bqk_sb = consts.tile([128, 4], f32)
    nc.sync.dma_start(bqk_sb, d["bqk"])
    bp_sb = consts.tile([128, 2], f32)
    nc.sync.dma_start(bp_sb, d["bp"])
    ebrep_sb = consts.tile([128, 4, 512], bf16)
    nc.sync.dma_start(ebrep_sb, d["ebrep"])

    # ---- persistent xT (full residency) ----
    xT_sb = []
    for kc in range(2):
        t = persist.tile([128, TOK], bf16, name=f"xT_sb{kc}")
        # split the load into 4 DMAs for parallelism
        for i in range(4):
            sl = slice(i * TOK // 4, (i + 1) * TOK // 4)
            nc.sync.dma_start(t[:, sl], d["xT"][kc, :, sl])
        xT_sb.append(t)

    # ---- persistent V tiles (ping-pong by round parity), ones pre-set ----
    v_sb = []
    for par in range(2):
        t = persist.tile([128, 2048], bf16, name=f"v_sb{par}")
        nc.vector.memset(
            t.rearrange("p (u h e) -> p u h e", u=4, h=8)[:, :, :, 32:64], 1.0
        )
        v_sb.append(t)

    def bcast_free(ap, n):
        new = bass.AP(ap.tensor, ap.offset, [list(x) for x in ap.ap] + [[0, n]])
        return new

    for R in range(ROUNDS):
        tok0 = 512 * R
        par = R % 2

        # ================= QKV projections =================
        q_sb, k_sb = [], []
        for fc in range(2):
            qps = ps_mm.tile([128, 512], f32, name="mmps", tag="mmps")
            for kc in range(2):
                nc.tensor.matmul(
                    qps, wqk_sb[:, kc, 128 * fc: 128 * (fc + 1)],
                    xT_sb[kc][:, tok0: tok0 + 512],
                    start=(kc == 0), stop=(kc == 1),
                )
            qt = sb_qk.tile([128, 512], bf16, name=f"q_sb{fc}", tag=f"q{fc}")
            nc.scalar.activation(qt, qps, AFT.Identity, bias=bqk_sb[:, fc: fc + 1])
            q_sb.append(qt)
        for fc in range(2):
            kps = ps_mm.tile([128, 512], f32, name="mmps", tag="mmps")
            for kc in range(2):
                nc.tensor.matmul(
                    kps, wqk_sb[:, kc, 256 + 128 * fc: 256 + 128 * (fc + 1)],
                    xT_sb[kc][:, tok0: tok0 + 512],
                    start=(kc == 0), stop=(kc == 1),
                )
            kt = sb_qk.tile([128, 512], bf16, name=f"k_sb{fc}", tag=f"k{fc}")
            nc.vector.tensor_scalar_add(kt, kps, bqk_sb[:, 2 + fc: 3 + fc])
            k_sb.append(kt)
        # V: tokens on partitions; out [tok128, 256] pairs per psum bank
        for vf in range(2):
            vps = ps_mm.tile([128, 512], f32, name="mmps", tag="mmps")
            for tcix in range(2):
                t0 = tok0 + 256 * vf + 128 * tcix
                for kc in range(2):
                    nc.tensor.matmul(
                        vps[:, 256 * tcix: 256 * (tcix + 1)],
                        xT_sb[kc][:, t0: t0 + 128],
                        wv_sb[:, kc, :],
                        start=(kc == 0), stop=(kc == 1),
                    )
            # scatter-copy into v_sb[par] at [tok, 64h + d]
            src = vps.rearrange("p (tc h e) -> p tc h e", tc=2, h=8)
            dst = v_sb[par][:, 1024 * vf: 1024 * (vf + 1)].rearrange(
                "p (tc h e) -> p tc h e", tc=2, h=8
            )[:, :, :, 0:32]
            nc.scalar.activation(dst, src, AFT.Copy)

        # ================= S^T = K Q^T per (pos, head) =================
        sps = [ps_s.tile([128, 512], f32, name=f"sps{r}", tag=f"s{r}") for r in range(4)]
        for s in range(8):
            c = s % 2
            for h in range(HEADS):
                r, hc, hh = h % 4, h // 4, h // 4
                hr = 32 * (h % 4)
                slot = 4 * hh + s // 2
                nc.tensor.matmul(
                    sps[r][64 * c: 64 * (c + 1), 64 * slot: 64 * (slot + 1)],
                    k_sb[hc][hr: hr + 32, 64 * s: 64 * (s + 1)],
                    q_sb[hc][hr: hr + 32, 64 * s: 64 * (s + 1)],
                    start=True, stop=True,
                    tile_position=(hr, 64 * c),
                )

        # exp + bias/mask multiply
        p_sb = []
        for r in range(4):
            et = sb_ep.tile([128, 512], bf16, name=f"e_sb{r}", tag="esb")
            nc.scalar.activation(et, sps[r], AFT.Exp)
            pt = sb_ep.tile([128, 512], bf16, name=f"p_sb{r}", tag="psb")
            nc.vector.tensor_mul(pt, et, ebrep_sb[:, r, :])
            p_sb.append(pt)

        # ================= AV + denom, normalize =================
        avn = sb.tile([128, 1024], bf16, name="avn", tag="avn")
        for r in range(4):
            avps = ps_av.tile([128, 512], f32, name="avps", tag="avps")
            for s in range(8):
                a = s % 2
                for hh in range(2):
                    h = r + 4 * hh
                    b = 2 * (s // 2) + hh
                    slot = 4 * hh + s // 2
                    nc.tensor.matmul(
                        avps[64 * a: 64 * (a + 1), 64 * b: 64 * b + 33],
                        p_sb[r][64 * a: 64 * (a + 1), 64 * slot: 64 * (slot + 1)],
                        v_sb[par][64 * a: 64 * (a + 1),
                                  512 * (s // 2) + 64 * h: 512 * (s // 2) + 64 * h + 33],
                        start=True, stop=True,
                        tile_position=(64 * a, 64 * a),
                    )
            rsb = sb_ep.tile([128, 8], f32, name="rsb", tag="rsb")
            nc.vector.reciprocal(
                rsb, avps.rearrange("p (b e) -> p b e", b=8)[:, :, 32]
            )
            nc.vector.tensor_mul(
                avn.rearrange("p (b q e) -> p b q e", b=8, q=4)[:, :, r, :],
                avps.rearrange("p (b e) -> p b e", b=8)[:, :, 0:32],
                bcast_free(rsb, 32),
            )

        # ================= transpose + output projection =================
        avt = [sb.tile([128, 512], bf16, name=f"avt{w}", tag=f"avt{w}") for w in range(2)]
        for g in range(8):
            w, u = g % 2, g // 2
            nc.sync.dma_start_transpose(
                avt[w][:, 128 * u: 128 * (u + 1)],
                avn[:, 128 * g: 128 * (g + 1)],
            )
        for ec in range(2):
            pps = ps_mm.tile([128, 512], f32, name="mmps", tag="mmps")
            for w in range(2):
                nc.tensor.matmul(
                    pps, wp_sb[:, w, 128 * ec: 128 * (ec + 1)], avt[w],
                    start=(w == 0), stop=(w == 1),
                )
            osb = sb.tile([128, 512], f32, name="osb", tag=f"osb{ec}")
            nc.vector.tensor_scalar_add(osb, pps, bp_sb[:, ec: ec + 1])
            nc.sync.dma_start(d["outT"][128 * ec: 128 * (ec + 1), tok0: tok0 + 512], osb)


def build():
    """Build + compile the Bass program (cached)."""
    if "nc" in _BUILD_CACHE:
        return _BUILD_CACHE["nc"]
    import concourse.bass as bass
    import concourse.mybir as mybir
    import concourse.tile as tile
    from concourse import bacc

    f32 = mybir.dt.float32
    bf16 = mybir.dt.bfloat16

    nc = bacc.Bacc("TRN2", target_bir_lowering=False, debug=False,
                   enable_asserts=False, num_devices=NCORES)
    d = {
        "xT": nc.dram_tensor("xT", [2, 128, TOK], bf16, kind="ExternalInput").ap(),
        "wqk": nc.dram_tensor("wqk", [128, 2, 512], bf16, kind="ExternalInput").ap(),
        "wv": nc.dram_tensor("wv", [128, 2, 256], bf16, kind="ExternalInput").ap(),
        "wp": nc.dram_tensor("wp", [128, 2, 256], bf16, kind="ExternalInput").ap(),
        "bqk": nc.dram_tensor("bqk", [128, 4], f32, kind="ExternalInput").ap(),
        "bp": nc.dram_tensor("bp", [128, 2], f32, kind="ExternalInput").ap(),
        "ebrep": nc.dram_tensor("ebrep", [128, 4, 512], bf16, kind="ExternalInput").ap(),
        "outT": nc.dram_tensor("outT", [256, TOK], f32, kind="ExternalOutput").ap(),
    }
    with tile.TileContext(nc) as tc:
        with ExitStack() as es:
            tc._emit_ctx = es
            _emit(nc, tc, d)
    nc.compile()
    _BUILD_CACHE["nc"] = nc
    return nc


def _install_ntff_hook():
    """Provide antenv.axon_hooks with a ctypes NTFF profiling hook if the
    image's antenv package lacks it (mirrors the agent-boot registration)."""
    import contextlib
    import ctypes
    import types

    try:
        from antenv.axon_hooks import get_axon_ntff_profile_hook  # noqa: F401
        return True
    except ImportError:
        pass
    so_path = "/opt/axon/libaxon_pjrt.so"
    if not os.path.exists(so_path):
        return False
    lib = ctypes.CDLL(so_path)
    if not hasattr(lib, "axon_start_nrt_profile"):
        return False
    lib.axon_start_nrt_profile.argtypes = [ctypes.POINTER(ctypes.c_int64), ctypes.c_size_t]
    lib.axon_start_nrt_profile.restype = ctypes.c_int64
    lib.axon_stop_nrt_profile.argtypes = [ctypes.c_char_p]
    lib.axon_stop_nrt_profile.restype = ctypes.c_int64

    @contextlib.contextmanager
    def _hook(output_dir, device_ids):
        import jax
        jax.devices()
        if device_ids:
            ids = (ctypes.c_int64 * len(device_ids))(*device_ids)
            rc = lib.axon_start_nrt_profile(ids, len(device_ids))
        else:
            rc = lib.axon_start_nrt_profile(None, 0)
        if rc != 0:
            raise RuntimeError(f"axon_start_nrt_profile rc={rc}")
        try:
            yield
        finally:
            n = lib.axon_stop_nrt_profile(str(output_dir).encode())
            print(f"profile: {n} file(s) written to {output_dir}", file=sys.stderr)

    import antenv
    mod = types.ModuleType("antenv.axon_hooks")
    _state = {"hook": _hook}
    mod.get_axon_ntff_profile_hook = lambda: _state["hook"]
    mod.set_axon_ntff_profile_hook = lambda h: _state.update(hook=h)
    sys.modules["antenv.axon_hooks"] = mod
    antenv.axon_hooks = mod
    return True


def kernel(x, rpb_table, qkv_w, qkv_b, proj_w, proj_b):
    in_maps = _host_prep(x, rpb_table, qkv_w, qkv_b, proj_w, proj_b)
    nc = build()
    from concourse import bass_utils

    trace = bool(int(os.environ.get("BASS_KERNEL_TRACE", "0")))
    if trace:
        trace = _install_ntff_hook()
    try:
        res = bass_utils.run_bass_kernel_spmd(
            nc, in_maps, core_ids=list(range(NCORES)), trace=trace
        )
    except Exception:
        if not trace:
            raise
        import traceback
        traceback.print_exc()
        print("trace run failed; retrying without trace", file=sys.stderr)
        res = bass_utils.run_bass_kernel_spmd(
            nc, in_maps, core_ids=list(range(NCORES)), trace=False
        )
    if trace and res.exec_time_ns is not None:
        print(f"HW exec time: {res.exec_time_ns} ns")
        _BUILD_CACHE["exec_time_ns"] = res.exec_time_ns
        _BUILD_CACHE["profile_res"] = res

    x = np.asarray(x, dtype=np.float32)
    out = np.empty_like(x)
    out[:, :, :HALF] = x[:, :, :HALF]
    # outT per core: [256, 9216] -> positions
    attn = np.empty((HALF, NPOS, T), dtype=np.float32)
    for m in range(NCORES):
        o = res.results[m]["outT"]
        attn[:, m * POS_PER_CORE: (m + 1) * POS_PER_CORE, :] = o.reshape(
            HALF, POS_PER_CORE, T
        )
    # (c, B, H, W, T) -> (B, T, c, H, W)
    attn = attn.reshape(HALF, B, HS, WS, T)
    out[:, :, HALF:] = np.transpose(attn, (1, 4, 0, 2, 3))
    return out


# revision 12
# speedup vs baseline: 2.1222x; 2.1222x over previous
"""Trainium2 Bass kernel for nn_BottleneckBlock (half-channel causal attention).

Contract: kernel(**inputs) takes the FULL unsharded inputs (as produced by the
problem's setup_inputs) and returns the FULL output, distributing work across
8 NeuronCores data-parallel over the (b, h, w) positions.

Per-core math (144 positions, seq N=64, C=256, 8 heads x 32):
  qkv = x @ qkv_w.T + qkv_b ; causal softmax(q k^T / sqrt(32) + rpb) @ v ; proj.
Layout strategy: everything token-transposed ([channel, token] in SBUF) so that
matmul contractions always run along partitions. The softmax bias+mask is
applied multiplicatively as exp(bias) (0 above the diagonal); the softmax
denominator comes out of the attention-value matmul via an appended
all-ones column; V's projection bias is folded into the output projection
bias (b_out' = proj_b + proj_w @ b_v) on the host.
"""

import os
import sys
from contextlib import ExitStack

import numpy as np

sys.path.insert(0, "/opt/trn_rl_repo")

import ml_dtypes

BF16 = ml_dtypes.bfloat16

# Problem shape (hardcoded per spec)
B, T, CH, HS, WS = 2, 64, 512, 24, 24
HALF = CH // 2          # 256
HEADS = 8
HD = 32
SCALE = HD ** -0.5
NCORES = 8
NPOS = B * HS * WS      # 1152 positions
POS_PER_CORE = NPOS // NCORES   # 144
TOK = POS_PER_CORE * T  # 9216 tokens per core
ROUNDS = TOK // 512     # 18 rounds of 8 positions

_BUILD_CACHE = {}


def _bf(a):
    return np.ascontiguousarray(a.astype(BF16))


def _host_prep(x, rpb_table, qkv_w, qkv_b, proj_w, proj_b):
    """Build the 8 per-core input maps + keep what's needed for reassembly."""
    x = np.asarray(x, dtype=np.float32)
    qkv_w = np.asarray(qkv_w, dtype=np.float32)
    qkv_b = np.asarray(qkv_b, dtype=np.float32)
    proj_w = np.asarray(proj_w, dtype=np.float32)
    proj_b = np.asarray(proj_b, dtype=np.float32)
    rpb = np.asarray(rpb_table, dtype=np.float32)

    # ---- x transpose: (B,T,CH,H,W) attention half -> [c, B_*T] ----
    b_part = x[:, :, HALF:]                       # (B,T,256,H,W)
    xt = np.transpose(b_part, (2, 0, 3, 4, 1))    # (256, B, H, W, T)
    xt = np.ascontiguousarray(xt).reshape(HALF, NPOS * T)
    xt_bf = xt.astype(BF16)

    # ---- weights ----
    # wqk[p, kc, f]: f in [0,512) = (q heads then k heads); q rows pre-scaled
    wqk = np.empty((128, 2, 512), dtype=np.float32)
    for kc in range(2):
        wqk[:, kc, 0:256] = (qkv_w[0:256] * SCALE).T[128 * kc: 128 * (kc + 1)]
        wqk[:, kc, 256:512] = qkv_w[256:512].T[128 * kc: 128 * (kc + 1)]
    bqk = np.stack(
        [qkv_b[0:128] * SCALE, qkv_b[128:256] * SCALE,
         qkv_b[256:384], qkv_b[384:512]], axis=1,
    ).astype(np.float32)                          # [128, 4]

    wv = np.empty((128, 2, 256), dtype=np.float32)
    for kc in range(2):
        wv[:, kc, :] = qkv_w[512:768].T[128 * kc: 128 * (kc + 1)]

    wp = np.empty((128, 2, 256), dtype=np.float32)
    for w in range(2):
        wp[:, w, :] = proj_w.T[128 * w: 128 * (w + 1)]
    bv = qkv_b[512:768]
    bp_full = proj_b + proj_w @ bv
    bp = np.stack([bp_full[0:128], bp_full[128:256]], axis=1).astype(np.float32)

    # ---- exp(bias)+causal-mask tiles, transposed & replicated ----
    pos = np.arange(T)
    rel = pos[None, :] - pos[:, None] + (T - 1)   # [i, j]
    bias = rpb[rel]                               # [i, j, heads]
    eb = np.exp(bias.transpose(2, 0, 1))          # [h, i, j]
    eb = eb * (pos[None, None, :] <= pos[None, :, None])  # zero j>i
    ebT = np.transpose(eb, (0, 2, 1))             # [h, j, i]
    # bank r = h%4; free slot = 4*(h//4) + (s//2) -> head h = r + 4*(slot//4)
    ebrep = np.empty((128, 4, 512), dtype=np.float32)
    for r in range(4):
        for q in range(2):
            for slot in range(8):
                h = r + 4 * (slot // 4)
                ebrep[64 * q: 64 * (q + 1), r, 64 * slot: 64 * (slot + 1)] = ebT[h]

    ident = np.eye(128, dtype=np.float32)
    wqk_bf, wv_bf, wp_bf, ebrep_bf, ident_bf = (
        _bf(wqk), _bf(wv), _bf(wp), _bf(ebrep), _bf(ident))

    in_maps = []
    for m in range(NCORES):
        sl = xt_bf[:, m * TOK: (m + 1) * TOK]     # [256, 9216]
        in_maps.append({
            "xT": np.ascontiguousarray(sl.reshape(2, 128, TOK)),
            "wqk": wqk_bf,
            "wv": wv_bf,
            "wp": wp_bf,
            "bqk": bqk,
            "bp": bp,
            "ebrep": ebrep_bf,
            "ident": ident_bf,
        })
    return in_maps


def _emit(nc, tc, d):
    """Emit the Tile kernel. d: dict of dram APs."""
    import concourse.bass as bass
    import concourse.mybir as mybir

    f32 = mybir.dt.float32
    bf16 = mybir.dt.bfloat16
    AFT = mybir.ActivationFunctionType

    ctx = tc._emit_ctx  # ExitStack installed by caller

    consts = ctx.enter_context(tc.tile_pool(name="consts", bufs=1))
    persist = ctx.enter_context(tc.tile_pool(name="persist", bufs=1))
    sb = ctx.enter_context(tc.tile_pool(name="sb", bufs=3))
    sb_qk = ctx.enter_context(tc.tile_pool(name="sb_qk", bufs=3))
    sb_ep = ctx.enter_context(tc.tile_pool(name="sb_ep", bufs=6))
    ps_qkv = ctx.enter_context(tc.tile_pool(name="ps_qkv", bufs=2, space="PSUM"))
    ps_s = ctx.enter_context(tc.tile_pool(name="ps_s", bufs=1, space="PSUM"))
    ps_av = ctx.enter_context(tc.tile_pool(name="ps_av", bufs=2, space="PSUM"))
    ps_pt = ctx.enter_context(tc.tile_pool(name="ps_pt", bufs=2, space="PSUM"))

    # ---- load constants ----
    wqk_sb = consts.tile([128, 2, 512], bf16)
    nc.sync.dma_start(wqk_sb, d["wqk"])
    wv_sb = consts.tile([128, 2, 256], bf16)
    nc.sync.dma_start(wv_sb, d["wv"])
    wp_sb = consts.tile([128, 2, 256], bf16)
    nc.sync.dma_start(wp_sb, d["wp"])
    bqk_sb = consts.tile([128, 4], f32)
    nc.sync.dma_start(bqk_sb, d["bqk"])
    bp_sb = consts.tile([128, 2], f32)
    nc.sync.dma_start(bp_sb, d["bp"])
    ebrep_sb = consts.tile([128, 4, 512], bf16)
    nc.sync.dma_start(ebrep_sb, d["ebrep"])
    ident_sb = consts.tile([128, 128], bf16)
    nc.sync.dma_start(ident_sb, d["ident"])

    # ---- persistent xT (full residency) ----
    xT_sb = []
    for kc in range(2):
        t = persist.tile([128, TOK], bf16, name=f"xT_sb{kc}")
        for i in range(4):
            sl = slice(i * TOK // 4, (i + 1) * TOK // 4)
            nc.sync.dma_start(t[:, sl], d["xT"][kc, :, sl])
        xT_sb.append(t)

    # ---- persistent V ([tok-chunk, head-slot 33 = 32 v + 1 one]) ----
    v_all = persist.tile([128, TOK // 128, 320], bf16, name="v_all")
    nc.vector.memset(
        v_all.rearrange("p u (h e) -> p u h e", h=8)[:, :, :, 32:40], 1.0
    )

    def bcast_free(ap, n):
        return bass.AP(ap.tensor, ap.offset, [list(x) for x in ap.ap] + [[0, n]])

    NR = TOK // 512  # 18 rounds of 8 positions
    for R in range(NR):
        tok0 = 512 * R

        # ================= QKV projections =================
        q_sb, k_sb = [], []
        for fc in range(2):
            qps = ps_qkv.tile([128, 512], f32, name="qps", tag="mmps")
            for kc in range(2):
                nc.tensor.matmul(
                    qps, wqk_sb[:, kc, 128 * fc: 128 * (fc + 1)],
                    xT_sb[kc][:, tok0: tok0 + 512],
                    start=(kc == 0), stop=(kc == 1),
                )
            qt = sb_qk.tile([128, 512], bf16, name=f"q_sb{fc}", tag=f"q{fc}")
            nc.scalar.activation(qt, qps, AFT.Identity, bias=bqk_sb[:, fc: fc + 1])
            q_sb.append(qt)
        for fc in range(2):
            kps = ps_qkv.tile([128, 512], f32, name="kps", tag="mmps")
            for kc in range(2):
                nc.tensor.matmul(
                    kps, wqk_sb[:, kc, 256 + 128 * fc: 256 + 128 * (fc + 1)],
                    xT_sb[kc][:, tok0: tok0 + 512],
                    start=(kc == 0), stop=(kc == 1),
                )
            kt = sb_qk.tile([128, 512], bf16, name=f"k_sb{fc}", tag=f"k{fc}")
            nc.vector.tensor_scalar_add(kt, kps, bqk_sb[:, 2 + fc: 3 + fc])
            k_sb.append(kt)
        for vf in range(2):
            vps = ps_qkv.tile([128, 512], f32, name="vps", tag="mmps")
            for tcix in range(2):
                t0 = tok0 + 256 * vf + 128 * tcix
                for kc in range(2):
                    nc.tensor.matmul(
                        vps[:, 256 * tcix: 256 * (tcix + 1)],
                        xT_sb[kc][:, t0: t0 + 128],
                        wv_sb[:, kc, :],
                        start=(kc == 0), stop=(kc == 1),
                    )
            src_v = vps.rearrange("p (tc h e) -> p tc h e", tc=2, h=8)
            dst_v = v_all.rearrange("p u (h e) -> p u h e", h=8)[
                :, 4 * R + 2 * vf: 4 * R + 2 * vf + 2, :, 0:32]
            nc.scalar.activation(dst_v, src_v, AFT.Copy)

        # ========== S^T = K Q^T: bank r=h%4 (one row-group per bank) ==========
        sps = [ps_s.tile([128, 512], f32, name=f"sps{r}", tag=f"s{r}") for r in range(4)]
        for s in range(8):
            c = s % 2
            for h in range(HEADS):
                r = h % 4
                slot = 4 * (h // 4) + s // 2
                hr = 32 * (h % 4)
                nc.tensor.matmul(
                    sps[r][64 * c: 64 * (c + 1), 64 * slot: 64 * (slot + 1)],
                    k_sb[h // 4][hr: hr + 32, 64 * s: 64 * (s + 1)],
                    q_sb[h // 4][hr: hr + 32, 64 * s: 64 * (s + 1)],
                    start=True, stop=True,
                    tile_position=(hr, 64 * c),
                )
        p_sb = []
        for r in range(4):
            et = sb_ep.tile([128, 512], bf16, name=f"e_sb{r}", tag="esb")
            nc.scalar.activation(et, sps[r], AFT.Exp)
            pt = sb_ep.tile([128, 512], bf16, name=f"p_sb{r}", tag="psb")
            nc.vector.tensor_mul(pt, et, ebrep_sb[:, r, :])
            p_sb.append(pt)

        # ============ AV + denom + normalize (banks shared with S) ============
        avn = sb.tile([128, 1024], bf16, name="avn", tag="avn")
        for r in range(4):
            avps = ps_s.tile([128, 512], f32, name=f"avps{r}", tag=f"s{r}")
            for s in range(8):
                a = s % 2
                for hh in range(2):
                    h = r + 4 * hh
                    b = 2 * (s // 2) + hh
                    slot = 4 * hh + s // 2
                    nc.tensor.matmul(
                        avps[64 * a: 64 * (a + 1), 64 * b: 64 * b + 33],
                        p_sb[r][64 * a: 64 * (a + 1), 64 * slot: 64 * (slot + 1)],
                        v_all[64 * a: 64 * (a + 1), 4 * R + s // 2, 40 * h: 40 * h + 33],
                        start=True, stop=True,
                        tile_position=(64 * a, 64 * a),
                    )
            rsb = sb_ep.tile([128, 8], f32, name="rsb", tag="rsb")
            nc.vector.reciprocal(
                rsb, avps.rearrange("p (b e) -> p b e", b=8)[:, :, 32]
            )
            nc.vector.tensor_mul(
                avn.rearrange("p (b q e) -> p b q e", b=8, q=4)[:, :, r, :],
                avps.rearrange("p (b e) -> p b e", b=8)[:, :, 0:32],
                bcast_free(rsb, 32),
            )

        # ============ transpose via PE (x identity) + projection ============
        avt = [sb.tile([128, 4, 128], bf16, name=f"avt{w}", tag=f"avt{w}") for w in range(2)]
        for half in range(2):
            tps = ps_pt.tile([128, 4, 128], f32, name="tps", tag="ptps")
            for g4 in range(4):
                g = 4 * half + g4
                nc.tensor.matmul(
                    tps[:, g4, :], avn[:, 128 * g: 128 * (g + 1)], ident_sb,
                    start=True, stop=True,
                )
            for w in range(2):
                nc.scalar.activation(
                    avt[w][:, 2 * half: 2 * half + 2, :],
                    tps.rearrange("p (u w) f -> p u w f", w=2)[:, :, w, :],
                    AFT.Copy,
                )
        for ec in range(2):
            pps = ps_pt.tile([128, 512], f32, name="pps", tag="ptps")
            for w in range(2):
                nc.tensor.matmul(
                    pps, wp_sb[:, w, 128 * ec: 128 * (ec + 1)],
                    avt[w].rearrange("p u f -> p (u f)"),
                    start=(w == 0), stop=(w == 1),
                )
            osb = sb.tile([128, 512], f32, name="osb", tag=f"osb{ec}")
            nc.vector.tensor_scalar_add(osb, pps, bp_sb[:, ec: ec + 1])
            nc.sync.dma_start(
                d["outT"][128 * ec: 128 * (ec + 1), tok0: tok0 + 512], osb
            )


def build():
    """Build + compile the Bass program (cached)."""
    if "nc" in _BUILD_CACHE:
        return _BUILD_CACHE["nc"]
    import concourse.bass as bass
    import concourse.mybir as mybir
    import concourse.tile as tile
    from concourse import bacc

    f32 = mybir.dt.float32
    bf16 = mybir.dt.bfloat16

    nc = bacc.Bacc("TRN2", target_bir_lowering=False, debug=False,
                   enable_asserts=False, num_devices=NCORES)
    d = {
        "xT": nc.dram_tensor("xT", [2, 128, TOK], bf16, kind="ExternalInput").ap(),
        "wqk": nc.dram_tensor("wqk", [128, 2, 512], bf16, kind="ExternalInput").ap(),
        "wv": nc.dram_tensor("wv", [128, 2, 256], bf16, kind="ExternalInput").ap(),
        "wp": nc.dram_tensor("wp", [128, 2, 256], bf16, kind="ExternalInput").ap(),
        "bqk": nc.dram_tensor("bqk", [128, 4], f32, kind="ExternalInput").ap(),
        "bp": nc.dram_tensor("bp", [128, 2], f32, kind="ExternalInput").ap(),
        "ebrep": nc.dram_tensor("ebrep", [128, 4, 512], bf16, kind="ExternalInput").ap(),
        "ident": nc.dram_tensor("ident", [128, 128], bf16, kind="ExternalInput").ap(),
        "outT": nc.dram_tensor("outT", [256, TOK], f32, kind="ExternalOutput").ap(),
    }
    with tile.TileContext(nc) as tc:
        with ExitStack() as es:
            tc._emit_ctx = es
            _emit(nc, tc, d)
    nc.compile()
    _BUILD_CACHE["nc"] = nc
    return nc


def _install_ntff_hook():
    """Provide antenv.axon_hooks with a ctypes NTFF profiling hook if the
    image's antenv package lacks it (mirrors the agent-boot registration)."""
    import contextlib
    import ctypes
    import types

    try:
        from antenv.axon_hooks import get_axon_ntff_profile_hook  # noqa: F401
        return True
    except ImportError:
        pass
    so_path = "/opt/axon/libaxon_pjrt.so"
    if not os.path.exists(so_path):
        return False
    lib = ctypes.CDLL(so_path)
    if not hasattr(lib, "axon_start_nrt_profile"):
        return False
    lib.axon_start_nrt_profile.argtypes = [ctypes.POINTER(ctypes.c_int64), ctypes.c_size_t]
    lib.axon_start_nrt_profile.restype = ctypes.c_int64
    lib.axon_stop_nrt_profile.argtypes = [ctypes.c_char_p]
    lib.axon_stop_nrt_profile.restype = ctypes.c_int64

    @contextlib.contextmanager
    def _hook(output_dir, device_ids):
        import jax
        jax.devices()
        if device_ids:
            ids = (ctypes.c_int64 * len(device_ids))(*device_ids)
            rc = lib.axon_start_nrt_profile(ids, len(device_ids))
        else:
            rc = lib.axon_start_nrt_profile(None, 0)
        if rc != 0:
            raise RuntimeError(f"axon_start_nrt_profile rc={rc}")
        try:
            yield
        finally:
            n = lib.axon_stop_nrt_profile(str(output_dir).encode())
            print(f"profile: {n} file(s) written to {output_dir}", file=sys.stderr)

    import antenv
    mod = types.ModuleType("antenv.axon_hooks")
    _state = {"hook": _hook}
    mod.get_axon_ntff_profile_hook = lambda: _state["hook"]
    mod.set_axon_ntff_profile_hook = lambda h: _state.update(hook=h)
    sys.modules["antenv.axon_hooks"] = mod
    antenv.axon_hooks = mod
    return True


def kernel(x, rpb_table, qkv_w, qkv_b, proj_w, proj_b):
    in_maps = _host_prep(x, rpb_table, qkv_w, qkv_b, proj_w, proj_b)
    nc = build()
    from concourse import bass_utils

    trace = bool(int(os.environ.get("BASS_KERNEL_TRACE", "0")))
    if trace:
        trace = _install_ntff_hook()
    try:
        res = bass_utils.run_bass_kernel_spmd(
            nc, in_maps, core_ids=list(range(NCORES)), trace=trace
        )
    except Exception:
        if not trace:
            raise
        import traceback
        traceback.print_exc()
        print("trace run failed; retrying without trace", file=sys.stderr)
        res = bass_utils.run_bass_kernel_spmd(
            nc, in_maps, core_ids=list(range(NCORES)), trace=False
        )
    if trace and res.exec_time_ns is not None:
        print(f"HW exec time: {res.exec_time_ns} ns")
        _BUILD_CACHE["exec_time_ns"] = res.exec_time_ns
        _BUILD_CACHE["profile_res"] = res

    x = np.asarray(x, dtype=np.float32)
    out = np.empty_like(x)
    out[:, :, :HALF] = x[:, :, :HALF]
    # outT per core: [256, 9216] -> positions
    attn = np.empty((HALF, NPOS, T), dtype=np.float32)
    for m in range(NCORES):
        o = res.results[m]["outT"]
        attn[:, m * POS_PER_CORE: (m + 1) * POS_PER_CORE, :] = o.reshape(
            HALF, POS_PER_CORE, T
        )
    # (c, B, H, W, T) -> (B, T, c, H, W)
    attn = attn.reshape(HALF, B, HS, WS, T)
    out[:, :, HALF:] = np.transpose(attn, (1, 4, 0, 2, 3))
    return out


# revision 13
# speedup vs baseline: 2.1440x; 1.0103x over previous
"""Trainium2 Bass kernel for nn_BottleneckBlock (half-channel causal attention).

Contract: kernel(**inputs) takes the FULL unsharded inputs (as produced by the
problem's setup_inputs) and returns the FULL output, distributing work across
8 NeuronCores data-parallel over the (b, h, w) positions.

Per-core math (144 positions, seq N=64, C=256, 8 heads x 32):
  qkv = x @ qkv_w.T + qkv_b ; causal softmax(q k^T / sqrt(32) + rpb) @ v ; proj.
Layout strategy: everything token-transposed ([channel, token] in SBUF) so that
matmul contractions always run along partitions. The softmax bias+mask is
applied multiplicatively as exp(bias) (0 above the diagonal); the softmax
denominator comes out of the attention-value matmul via an appended
all-ones column; V's projection bias is folded into the output projection
bias (b_out' = proj_b + proj_w @ b_v) on the host.
"""

import os
import sys
from contextlib import ExitStack

import numpy as np

sys.path.insert(0, "/opt/trn_rl_repo")

import ml_dtypes

BF16 = ml_dtypes.bfloat16

# Problem shape (hardcoded per spec)
B, T, CH, HS, WS = 2, 64, 512, 24, 24
HALF = CH // 2          # 256
HEADS = 8
HD = 32
SCALE = HD ** -0.5
NCORES = 8
NPOS = B * HS * WS      # 1152 positions
POS_PER_CORE = NPOS // NCORES   # 144
TOK = POS_PER_CORE * T  # 9216 tokens per core
ROUNDS = TOK // 512     # 18 rounds of 8 positions

_BUILD_CACHE = {}


def _bf(a):
    return np.ascontiguousarray(a.astype(BF16))


def _host_prep(x, rpb_table, qkv_w, qkv_b, proj_w, proj_b):
    """Build the 8 per-core input maps + keep what's needed for reassembly."""
    x = np.asarray(x, dtype=np.float32)
    qkv_w = np.asarray(qkv_w, dtype=np.float32)
    qkv_b = np.asarray(qkv_b, dtype=np.float32)
    proj_w = np.asarray(proj_w, dtype=np.float32)
    proj_b = np.asarray(proj_b, dtype=np.float32)
    rpb = np.asarray(rpb_table, dtype=np.float32)

    # ---- x transpose: (B,T,CH,H,W) attention half -> [c, B_*T] ----
    b_part = x[:, :, HALF:]                       # (B,T,256,H,W)
    xt = np.transpose(b_part, (2, 0, 3, 4, 1))    # (256, B, H, W, T)
    xt = np.ascontiguousarray(xt).reshape(HALF, NPOS * T)
    xt_bf = xt.astype(BF16)

    # ---- weights ----
    # wqk[p, kc, f]: f in [0,512) = (q heads then k heads); q rows pre-scaled
    wqk = np.empty((128, 2, 512), dtype=np.float32)
    for kc in range(2):
        wqk[:, kc, 0:256] = (qkv_w[0:256] * SCALE).T[128 * kc: 128 * (kc + 1)]
        wqk[:, kc, 256:512] = qkv_w[256:512].T[128 * kc: 128 * (kc + 1)]
    bqk = np.stack(
        [qkv_b[0:128] * SCALE, qkv_b[128:256] * SCALE,
         qkv_b[256:384], qkv_b[384:512]], axis=1,
    ).astype(np.float32)                          # [128, 4]

    wv = np.empty((128, 2, 256), dtype=np.float32)
    for kc in range(2):
        wv[:, kc, :] = qkv_w[512:768].T[128 * kc: 128 * (kc + 1)]

    wp = np.empty((128, 2, 256), dtype=np.float32)
    for w in range(2):
        wp[:, w, :] = proj_w.T[128 * w: 128 * (w + 1)]
    bv = qkv_b[512:768]
    bp_full = proj_b + proj_w @ bv
    bp = np.stack([bp_full[0:128], bp_full[128:256]], axis=1).astype(np.float32)

    # ---- exp(bias)+causal-mask tiles, transposed & replicated ----
    pos = np.arange(T)
    rel = pos[None, :] - pos[:, None] + (T - 1)   # [i, j]
    bias = rpb[rel]                               # [i, j, heads]
    eb = np.exp(bias.transpose(2, 0, 1))          # [h, i, j]
    eb = eb * (pos[None, None, :] <= pos[None, :, None])  # zero j>i
    ebT = np.transpose(eb, (0, 2, 1))             # [h, j, i]
    # bank r = h%4; free slot = 4*(h//4) + (s//2) -> head h = r + 4*(slot//4)
    ebrep = np.empty((128, 4, 512), dtype=np.float32)
    for r in range(4):
        for q in range(2):
            for slot in range(8):
                h = r + 4 * (slot // 4)
                ebrep[64 * q: 64 * (q + 1), r, 64 * slot: 64 * (slot + 1)] = ebT[h]

    ident = np.eye(128, dtype=np.float32)
    wqk_bf, wv_bf, wp_bf, ebrep_bf, ident_bf = (
        _bf(wqk), _bf(wv), _bf(wp), _bf(ebrep), _bf(ident))

    in_maps = []
    for m in range(NCORES):
        sl = xt_bf[:, m * TOK: (m + 1) * TOK]     # [256, 9216]
        in_maps.append({
            "xT": np.ascontiguousarray(sl.reshape(2, 128, TOK)),
            "wqk": wqk_bf,
            "wv": wv_bf,
            "wp": wp_bf,
            "bqk": bqk,
            "bp": bp,
            "ebrep": ebrep_bf,
            "ident": ident_bf,
        })
    return in_maps


def _emit(nc, tc, d):
    """Emit the Tile kernel. d: dict of dram APs."""
    import concourse.bass as bass
    import concourse.mybir as mybir

    f32 = mybir.dt.float32
    bf16 = mybir.dt.bfloat16
    AFT = mybir.ActivationFunctionType

    ctx = tc._emit_ctx  # ExitStack installed by caller

    consts = ctx.enter_context(tc.tile_pool(name="consts", bufs=1))
    persist = ctx.enter_context(tc.tile_pool(name="persist", bufs=1))
    sb = ctx.enter_context(tc.tile_pool(name="sb", bufs=4))
    sb_qk = ctx.enter_context(tc.tile_pool(name="sb_qk", bufs=4))
    sb_ep = ctx.enter_context(tc.tile_pool(name="sb_ep", bufs=8))
    ps_qkv = ctx.enter_context(tc.tile_pool(name="ps_qkv", bufs=2, space="PSUM"))
    ps_s = ctx.enter_context(tc.tile_pool(name="ps_s", bufs=1, space="PSUM"))
    ps_av = ctx.enter_context(tc.tile_pool(name="ps_av", bufs=2, space="PSUM"))
    ps_pt = ctx.enter_context(tc.tile_pool(name="ps_pt", bufs=2, space="PSUM"))

    # ---- load constants ----
    wqk_sb = consts.tile([128, 2, 512], bf16)
    nc.sync.dma_start(wqk_sb, d["wqk"])
    wv_sb = consts.tile([128, 2, 256], bf16)
    nc.sync.dma_start(wv_sb, d["wv"])
    wp_sb = consts.tile([128, 2, 256], bf16)
    nc.sync.dma_start(wp_sb, d["wp"])
    bqk_sb = consts.tile([128, 4], f32)
    nc.sync.dma_start(bqk_sb, d["bqk"])
    bp_sb = consts.tile([128, 2], f32)
    nc.sync.dma_start(bp_sb, d["bp"])
    ebrep_sb = consts.tile([128, 4, 512], bf16)
    nc.sync.dma_start(ebrep_sb, d["ebrep"])
    ident_sb = consts.tile([128, 128], bf16)
    nc.sync.dma_start(ident_sb, d["ident"])

    # ---- persistent xT (full residency) ----
    xT_sb = []
    for kc in range(2):
        t = persist.tile([128, TOK], bf16, name=f"xT_sb{kc}")
        for i in range(4):
            sl = slice(i * TOK // 4, (i + 1) * TOK // 4)
            nc.sync.dma_start(t[:, sl], d["xT"][kc, :, sl])
        xT_sb.append(t)

    # ---- persistent V ([tok-chunk, head-slot 33 = 32 v + 1 one]) ----
    v_all = persist.tile([128, TOK // 128, 320], bf16, name="v_all")
    nc.vector.memset(
        v_all.rearrange("p u (h e) -> p u h e", h=8)[:, :, :, 32:40], 1.0
    )

    def bcast_free(ap, n):
        return bass.AP(ap.tensor, ap.offset, [list(x) for x in ap.ap] + [[0, n]])

    NR = TOK // 512  # 18 rounds of 8 positions
    for R in range(NR):
        tok0 = 512 * R

        # ================= QKV projections =================
        q_sb, k_sb = [], []
        for fc in range(2):
            qps = ps_qkv.tile([128, 512], f32, name="qps", tag="mmps")
            for kc in range(2):
                nc.tensor.matmul(
                    qps, wqk_sb[:, kc, 128 * fc: 128 * (fc + 1)],
                    xT_sb[kc][:, tok0: tok0 + 512],
                    start=(kc == 0), stop=(kc == 1),
                )
            qt = sb_qk.tile([128, 512], bf16, name=f"q_sb{fc}", tag=f"q{fc}")
            nc.scalar.activation(qt, qps, AFT.Identity, bias=bqk_sb[:, fc: fc + 1])
            q_sb.append(qt)
        for fc in range(2):
            kps = ps_qkv.tile([128, 512], f32, name="kps", tag="mmps")
            for kc in range(2):
                nc.tensor.matmul(
                    kps, wqk_sb[:, kc, 256 + 128 * fc: 256 + 128 * (fc + 1)],
                    xT_sb[kc][:, tok0: tok0 + 512],
                    start=(kc == 0), stop=(kc == 1),
                )
            kt = sb_qk.tile([128, 512], bf16, name=f"k_sb{fc}", tag=f"k{fc}")
            nc.vector.tensor_scalar_add(kt, kps, bqk_sb[:, 2 + fc: 3 + fc])
            k_sb.append(kt)
        for vf in range(2):
            vps = ps_qkv.tile([128, 512], f32, name="vps", tag="mmps")
            for tcix in range(2):
                t0 = tok0 + 256 * vf + 128 * tcix
                for kc in range(2):
                    nc.tensor.matmul(
                        vps[:, 256 * tcix: 256 * (tcix + 1)],
                        xT_sb[kc][:, t0: t0 + 128],
                        wv_sb[:, kc, :],
                        start=(kc == 0), stop=(kc == 1),
                    )
            src_v = vps.rearrange("p (tc h e) -> p tc h e", tc=2, h=8)
            dst_v = v_all.rearrange("p u (h e) -> p u h e", h=8)[
                :, 4 * R + 2 * vf: 4 * R + 2 * vf + 2, :, 0:32]
            nc.scalar.activation(dst_v, src_v, AFT.Copy)

        # ========== S^T = K Q^T: bank r=h%4 (one row-group per bank) ==========
        sps = [ps_s.tile([128, 512], f32, name=f"sps{r}", tag=f"s{r}") for r in range(4)]
        for s in range(8):
            c = s % 2
            for h in range(HEADS):
                r = h % 4
                slot = 4 * (h // 4) + s // 2
                hr = 32 * (h % 4)
                nc.tensor.matmul(
                    sps[r][64 * c: 64 * (c + 1), 64 * slot: 64 * (slot + 1)],
                    k_sb[h // 4][hr: hr + 32, 64 * s: 64 * (s + 1)],
                    q_sb[h // 4][hr: hr + 32, 64 * s: 64 * (s + 1)],
                    start=True, stop=True,
                    tile_position=(hr, 64 * c),
                )
        p_sb = []
        for r in range(4):
            et = sb_ep.tile([128, 512], bf16, name=f"e_sb{r}", tag="esb")
            nc.scalar.activation(et, sps[r], AFT.Exp)
            pt = sb_ep.tile([128, 512], bf16, name=f"p_sb{r}", tag="psb")
            nc.vector.tensor_mul(pt, et, ebrep_sb[:, r, :])
            p_sb.append(pt)

        # ============ AV + denom + normalize (banks shared with S) ============
        avn = sb.tile([128, 1024], bf16, name="avn", tag="avn")
        for r in range(4):
            avps = ps_s.tile([128, 512], f32, name=f"avps{r}", tag=f"s{r}")
            for s in range(8):
                a = s % 2
                for hh in range(2):
                    h = r + 4 * hh
                    b = 2 * (s // 2) + hh
                    slot = 4 * hh + s // 2
                    nc.tensor.matmul(
                        avps[64 * a: 64 * (a + 1), 64 * b: 64 * b + 33],
                        p_sb[r][64 * a: 64 * (a + 1), 64 * slot: 64 * (slot + 1)],
                        v_all[64 * a: 64 * (a + 1), 4 * R + s // 2, 40 * h: 40 * h + 33],
                        start=True, stop=True,
                        tile_position=(64 * a, 64 * a),
                    )
            rsb = sb_ep.tile([128, 8], f32, name="rsb", tag="rsb")
            nc.vector.reciprocal(
                rsb, avps.rearrange("p (b e) -> p b e", b=8)[:, :, 32]
            )
            nc.vector.tensor_mul(
                avn.rearrange("p (b q e) -> p b q e", b=8, q=4)[:, :, r, :],
                avps.rearrange("p (b e) -> p b e", b=8)[:, :, 0:32],
                bcast_free(rsb, 32),
            )

        # ============ transpose via PE (x identity) + projection ============
        avt = [sb.tile([128, 4, 128], bf16, name=f"avt{w}", tag=f"avt{w}") for w in range(2)]
        for half in range(2):
            tps = ps_pt.tile([128, 4, 128], f32, name="tps", tag="ptps")
            for g4 in range(4):
                g = 4 * half + g4
                nc.tensor.matmul(
                    tps[:, g4, :], avn[:, 128 * g: 128 * (g + 1)], ident_sb,
                    start=True, stop=True,
                )
            for w in range(2):
                nc.scalar.activation(
                    avt[w][:, 2 * half: 2 * half + 2, :],
                    tps.rearrange("p (u w) f -> p u w f", w=2)[:, :, w, :],
                    AFT.Copy,
                )
        for ec in range(2):
            pps = ps_pt.tile([128, 512], f32, name="pps", tag="ptps")
            for w in range(2):
                nc.tensor.matmul(
                    pps, wp_sb[:, w, 128 * ec: 128 * (ec + 1)],
                    avt[w].rearrange("p u f -> p (u f)"),
                    start=(w == 0), stop=(w == 1),
                )
            osb = sb.tile([128, 512], f32, name="osb", tag=f"osb{ec}")
            nc.vector.tensor_scalar_add(osb, pps, bp_sb[:, ec: ec + 1])
            nc.sync.dma_start(
                d["outT"][128 * ec: 128 * (ec + 1), tok0: tok0 + 512], osb
            )


def build():
    """Build + compile the Bass program (cached)."""
    if "nc" in _BUILD_CACHE:
        return _BUILD_CACHE["nc"]
    import concourse.bass as bass
    import concourse.mybir as mybir
    import concourse.tile as tile
    from concourse import bacc

    f32 = mybir.dt.float32
    bf16 = mybir.dt.bfloat16

    nc = bacc.Bacc("TRN2", target_bir_lowering=False, debug=False,
                   enable_asserts=False, num_devices=NCORES)
    d = {
        "xT": nc.dram_tensor("xT", [2, 128, TOK], bf16, kind="ExternalInput").ap(),
        "wqk": nc.dram_tensor("wqk", [128, 2, 512], bf16, kind="ExternalInput").ap(),
        "wv": nc.dram_tensor("wv", [128, 2, 256], bf16, kind="ExternalInput").ap(),
        "wp": nc.dram_tensor("wp", [128, 2, 256], bf16, kind="ExternalInput").ap(),
        "bqk": nc.dram_tensor("bqk", [128, 4], f32, kind="ExternalInput").ap(),
        "bp": nc.dram_tensor("bp", [128, 2], f32, kind="ExternalInput").ap(),
        "ebrep": nc.dram_tensor("ebrep", [128, 4, 512], bf16, kind="ExternalInput").ap(),
        "ident": nc.dram_tensor("ident", [128, 128], bf16, kind="ExternalInput").ap(),
        "outT": nc.dram_tensor("outT", [256, TOK], f32, kind="ExternalOutput").ap(),
    }
    with tile.TileContext(nc) as tc:
        with ExitStack() as es:
            tc._emit_ctx = es
            _emit(nc, tc, d)
    nc.compile()
    _BUILD_CACHE["nc"] = nc
    return nc


def _install_ntff_hook():
    """Provide antenv.axon_hooks with a ctypes NTFF profiling hook if the
    image's antenv package lacks it (mirrors the agent-boot registration)."""
    import contextlib
    import ctypes
    import types

    try:
        from antenv.axon_hooks import get_axon_ntff_profile_hook  # noqa: F401
        return True
    except ImportError:
        pass
    so_path = "/opt/axon/libaxon_pjrt.so"
    if not os.path.exists(so_path):
        return False
    lib = ctypes.CDLL(so_path)
    if not hasattr(lib, "axon_start_nrt_profile"):
        return False
    lib.axon_start_nrt_profile.argtypes = [ctypes.POINTER(ctypes.c_int64), ctypes.c_size_t]
    lib.axon_start_nrt_profile.restype = ctypes.c_int64
    lib.axon_stop_nrt_profile.argtypes = [ctypes.c_char_p]
    lib.axon_stop_nrt_profile.restype = ctypes.c_int64

    @contextlib.contextmanager
    def _hook(output_dir, device_ids):
        import jax
        jax.devices()
        if device_ids:
            ids = (ctypes.c_int64 * len(device_ids))(*device_ids)
            rc = lib.axon_start_nrt_profile(ids, len(device_ids))
        else:
            rc = lib.axon_start_nrt_profile(None, 0)
        if rc != 0:
            raise RuntimeError(f"axon_start_nrt_profile rc={rc}")
        try:
            yield
        finally:
            n = lib.axon_stop_nrt_profile(str(output_dir).encode())
            print(f"profile: {n} file(s) written to {output_dir}", file=sys.stderr)

    import antenv
    mod = types.ModuleType("antenv.axon_hooks")
    _state = {"hook": _hook}
    mod.get_axon_ntff_profile_hook = lambda: _state["hook"]
    mod.set_axon_ntff_profile_hook = lambda h: _state.update(hook=h)
    sys.modules["antenv.axon_hooks"] = mod
    antenv.axon_hooks = mod
    return True


def kernel(x, rpb_table, qkv_w, qkv_b, proj_w, proj_b):
    in_maps = _host_prep(x, rpb_table, qkv_w, qkv_b, proj_w, proj_b)
    nc = build()
    from concourse import bass_utils

    trace = bool(int(os.environ.get("BASS_KERNEL_TRACE", "0")))
    if trace:
        trace = _install_ntff_hook()
    try:
        res = bass_utils.run_bass_kernel_spmd(
            nc, in_maps, core_ids=list(range(NCORES)), trace=trace
        )
    except Exception:
        if not trace:
            raise
        import traceback
        traceback.print_exc()
        print("trace run failed; retrying without trace", file=sys.stderr)
        res = bass_utils.run_bass_kernel_spmd(
            nc, in_maps, core_ids=list(range(NCORES)), trace=False
        )
    if trace and res.exec_time_ns is not None:
        print(f"HW exec time: {res.exec_time_ns} ns")
        _BUILD_CACHE["exec_time_ns"] = res.exec_time_ns
        _BUILD_CACHE["profile_res"] = res

    x = np.asarray(x, dtype=np.float32)
    out = np.empty_like(x)
    out[:, :, :HALF] = x[:, :, :HALF]
    # outT per core: [256, 9216] -> positions
    attn = np.empty((HALF, NPOS, T), dtype=np.float32)
    for m in range(NCORES):
        o = res.results[m]["outT"]
        attn[:, m * POS_PER_CORE: (m + 1) * POS_PER_CORE, :] = o.reshape(
            HALF, POS_PER_CORE, T
        )
    # (c, B, H, W, T) -> (B, T, c, H, W)
    attn = attn.reshape(HALF, B, HS, WS, T)
    out[:, :, HALF:] = np.transpose(attn, (1, 4, 0, 2, 3))
    return out
